# revision 4
# baseline (speedup 1.0000x reference)
"""Trainium2 Bass kernel for the teacher-forced/autoregressive tanh-RNN problem.

Contract: kernel(**inputs) takes FULL unsharded numpy inputs (as produced by
setup_inputs) and returns the FULL output [512, 1024, 16] float32.

Strategy (data-parallel over batch, 8 cores x 64 samples):
  - h kept in [H, B] layout (hidden dims on partitions, batch on free dim) so
    the sequential recurrence needs no transposes; weights are the stationary
    matmul operand.
  - Teacher-forced steps (t <= ps-1): only the recurrence runs per step
    (x-term matmuls with bias folded via an augmented ones-row + W_hh term,
    one fused tanh). FC head outputs are computed in batched matmuls per
    32-step chunk, interleaved into the PE stream as gap fillers.
  - Step t = ps: computed inline (produces r_ps needed for feedback).
  - Autoregressive steps (t > ps): feedback is folded through
    U = W_ih[:, :16] @ fc2_w (rank-16), so the critical cycle is
    tanh -> fc1 -> relu (DVE) -> U@r -> tanh. fc2/o_t is inline but off the
    critical path.
"""

import sys
import types

if "/opt/trn_rl_repo" not in sys.path:
    sys.path.insert(0, "/opt/trn_rl_repo")

import numpy as np

# Problem dims (hardcoded per the harness contract).
B_FULL, T, D = 512, 1024, 64
H, A, OD = 256, 256, 16
NCORES = 8
B = B_FULL // NCORES  # 64 per core

C1 = 32  # teacher-phase chunk (steps) for x staging + batched heads
C2 = 64  # autoregressive-phase chunk (steps) for x/o staging


def _install_axon_shims():
    """Make trace=True under axon survivable in this container (missing
    antenv.axon_hooks + no fish bucket for artifact upload). No-op effect
    when tracing is off."""
    try:
        import antenv  # noqa: F401
    except Exception:
        return
    if "antenv.axon_hooks" not in sys.modules:
        mod = types.ModuleType("antenv.axon_hooks")
        mod._hook = None

        def set_axon_ntff_profile_hook(h):
            mod._hook = h

        def get_axon_ntff_profile_hook():
            if mod._hook is None:
                try:
                    from trn_agent_boot.trn_boot import _ntff_profile_via_ctypes

                    mod._hook = _ntff_profile_via_ctypes("/opt/axon/libaxon_pjrt.so")
                except Exception:
                    return None
            return mod._hook

        mod.set_axon_ntff_profile_hook = set_axon_ntff_profile_hook
        mod.get_axon_ntff_profile_hook = get_axon_ntff_profile_hook
        sys.modules["antenv.axon_hooks"] = mod
        import antenv as _a

        _a.axon_hooks = mod
    try:
        import concourse.bass_utils as bu

        if not getattr(bu.upload_artifacts, "_is_local_stub", False):
            def _local_upload(tmpdir):
                return f"local:{tmpdir}"

            _local_upload._is_local_stub = True
            bu.upload_artifacts = _local_upload
    except Exception:
        pass


def _build_program(ps):
    """Trace + schedule + compile the per-core Bass program for a given
    prediction_start. Returns the compiled Bacc module."""
    import concourse.bass as bass  # noqa: F401
    import concourse.tile as tile
    from concourse import bacc, mybir

    f32 = mybir.dt.float32
    TANH = mybir.ActivationFunctionType.Tanh
    ADD = mybir.AluOpType.add
    MAX = mybir.AluOpType.max

    assert ps % C1 == 0 and (T - ps) % C2 == 0 and 0 < ps < T

    nc = bacc.Bacc("TRN2", target_bir_lowering=False, debug=False,
                   num_devices=NCORES)

    # ---- DRAM tensors ----
    xT = nc.dram_tensor("xT", [D, T * B], f32, kind="ExternalInput").ap()
    wih = nc.dram_tensor("wih", [D + 1, H], f32, kind="ExternalInput").ap()
    wtail = nc.dram_tensor("wtail", [D - OD + 1, H], f32,
                           kind="ExternalInput").ap()
    whhT = nc.dram_tensor("whhT", [128, 512], f32, kind="ExternalInput").ap()
    fc1T = nc.dram_tensor("fc1T", [128, 512], f32, kind="ExternalInput").ap()
    uT = nc.dram_tensor("uT", [128, 512], f32, kind="ExternalInput").ap()
    fc2T = nc.dram_tensor("fc2T", [128, 2 * OD], f32, kind="ExternalInput").ap()
    b1d = nc.dram_tensor("b1", [128, 2], f32, kind="ExternalInput").ap()
    b2d = nc.dram_tensor("b2", [OD, 1], f32, kind="ExternalInput").ap()
    o_out = nc.dram_tensor("o_out", [OD, T * B], f32, kind="ExternalOutput").ap()

    with tile.TileContext(nc) as tc:
        from contextlib import ExitStack

        outer = ExitStack()
        with outer:
            res = outer.enter_context(tc.tile_pool(name="res", bufs=1))
            ph = outer.enter_context(
                tc.tile_pool(name="ph", bufs=2, space="PSUM"))
            hpool = outer.enter_context(tc.tile_pool(name="hpool", bufs=3))
            rpool = outer.enter_context(tc.tile_pool(name="rpool", bufs=3))
            xpool = outer.enter_context(tc.tile_pool(name="xpool", bufs=2))
            oring2p = outer.enter_context(tc.tile_pool(name="oring2", bufs=2))

            # resident weights
            wih_s = res.tile([D + 1, H], f32)
            nc.sync.dma_start(out=wih_s, in_=wih)
            wtail_s = res.tile([D - OD + 1, H], f32)
            nc.sync.dma_start(out=wtail_s, in_=wtail)
            whh_s = res.tile([128, 512], f32)
            nc.sync.dma_start(out=whh_s, in_=whhT)
            fc1_s = res.tile([128, 512], f32)
            nc.sync.dma_start(out=fc1_s, in_=fc1T)
            u_s = res.tile([128, 512], f32)
            nc.sync.dma_start(out=u_s, in_=uT)
            fc2_s = res.tile([128, 2 * OD], f32)
            nc.sync.dma_start(out=fc2_s, in_=fc2T)
            b1_s = res.tile([128, 2], f32)
            nc.sync.dma_start(out=b1_s, in_=b1d)
            b2_s = res.tile([OD, 1], f32)
            nc.sync.dma_start(out=b2_s, in_=b2d)

            # ---- teacher-forced phase ----
            pending = []  # queued head-work closures (gap fillers for PE)

            def drain(k):
                for _ in range(min(k, len(pending))):
                    pending.pop(0)()

            def make_head_units(ring, rring, oring, t0, nsl):
                """Batched heads for one chunk: for each 512-col slice of
                8 steps: fc1 (per a-half) -> relu -> fc2 -> +b2 -> o ring."""
                units = []
                ring_r = ring.rearrange("p (s x) -> p s x", x=128)
                rring_r = rring.rearrange("p (s x) -> p s x", x=128)

                for n in range(nsl):
                    for m in range(2):
                        def u_fc1(n=n, m=m):
                            pa = pab.tile([128, 512], f32, tag="pa")
                            for kk in range(2):
                                nc.tensor.matmul(
                                    pa,
                                    lhsT=fc1_s[:, kk * 256 + m * 128:
                                               kk * 256 + (m + 1) * 128],
                                    rhs=ring_r[:, n * 8:(n + 1) * 8,
                                               kk * 64:(kk + 1) * 64],
                                    start=(kk == 0), stop=(kk == 1))
                            nc.vector.tensor_scalar(
                                rring_r[:, n * 8:(n + 1) * 8,
                                        m * 64:(m + 1) * 64],
                                pa.rearrange("p (s b) -> p s b", b=64),
                                b1_s[:, m:m + 1], 0.0, ADD, MAX)
                        units.append(u_fc1)

                    def u_fc2(n=n):
                        po = pob.tile([OD, 512], f32, tag="po")
                        for kk in range(2):
                            nc.tensor.matmul(
                                po,
                                lhsT=fc2_s[:, kk * OD:(kk + 1) * OD],
                                rhs=rring_r[:, n * 8:(n + 1) * 8,
                                            kk * 64:(kk + 1) * 64],
                                start=(kk == 0), stop=(kk == 1))
                        nc.vector.tensor_scalar_add(
                            oring[:, n * 512:(n + 1) * 512], po, b2_s)
                    units.append(u_fc2)

                def u_dma(t0=t0, oring=oring):
                    nc.sync.dma_start(
                        out=o_out[:, t0 * B:(t0 + C1) * B], in_=oring)
                units.append(u_dma)
                return units

            with tc.tile_pool(name="ring", bufs=2) as ringp, \
                 tc.tile_pool(name="rring", bufs=2) as rringp, \
                 tc.tile_pool(name="oring1", bufs=2) as oring1p, \
                 tc.tile_pool(name="pab", bufs=2, space="PSUM") as pab, \
                 tc.tile_pool(name="pob", bufs=2, space="PSUM") as pob:

                n_chunks = ps // C1
                prev_ring = None
                for ci in range(n_chunks):
                    t0 = ci * C1
                    xa = xpool.tile([D + 1, C1 * B], f32, tag="x1")
                    nc.sync.dma_start(out=xa[1:D + 1, :],
                                      in_=xT[:, t0 * B:(t0 + C1) * B])
                    nc.vector.memset(xa[0:1, :], 1.0)
                    ring = ringp.tile([128, C1 * 128], f32)

                    for j in range(C1):
                        t = t0 + j
                        psum = ph.tile([128, 128], f32)
                        xs = xa[0:D + 1, j * B:(j + 1) * B]
                        last = (t == 0)
                        for m in range(2):
                            nc.tensor.matmul(
                                psum[:, m * 64:(m + 1) * 64],
                                lhsT=wih_s[:, m * 128:(m + 1) * 128],
                                rhs=xs, start=(m == 0),
                                stop=(last and m == 1))
                        if t > 0:
                            hp = (ring[:, (j - 1) * 128:j * 128] if j > 0
                                  else prev_ring[:, (C1 - 1) * 128:C1 * 128])
                            for idx, (kk, m) in enumerate(
                                    ((0, 0), (1, 0), (0, 1), (1, 1))):
                                nc.tensor.matmul(
                                    psum[:, m * 64:(m + 1) * 64],
                                    lhsT=whh_s[:, kk * 256 + m * 128:
                                               kk * 256 + (m + 1) * 128],
                                    rhs=hp[:, kk * 64:(kk + 1) * 64],
                                    start=False, stop=(idx == 3))
                        nc.scalar.activation(
                            ring[:, j * 128:(j + 1) * 128], psum, TANH)
                        drain(1)

                    # queue this chunk's head work (drained during next chunk)
                    rring = rringp.tile([128, C1 * 128], f32)
                    oring = oring1p.tile([OD, C1 * B], f32)
                    pending.extend(
                        make_head_units(ring, rring, oring, t0,
                                        C1 * B // 512))
                    prev_ring = ring

                # ---- boundary step t = ps (inline head, teacher input) ----
                xps = xpool.tile([D + 1, B], f32, tag="xps")
                nc.sync.dma_start(out=xps[1:D + 1, :],
                                  in_=xT[:, ps * B:(ps + 1) * B])
                nc.vector.memset(xps[0:1, :], 1.0)

                psum = ph.tile([128, 128], f32)
                for m in range(2):
                    nc.tensor.matmul(
                        psum[:, m * 64:(m + 1) * 64],
                        lhsT=wih_s[:, m * 128:(m + 1) * 128],
                        rhs=xps, start=(m == 0), stop=False)
                for idx, (kk, m) in enumerate(((0, 0), (1, 0), (0, 1), (1, 1))):
                    nc.tensor.matmul(
                        psum[:, m * 64:(m + 1) * 64],
                        lhsT=whh_s[:, kk * 256 + m * 128:
                                   kk * 256 + (m + 1) * 128],
                        rhs=prev_ring[:, (C1 - 1) * 128 + kk * 64:
                                      (C1 - 1) * 128 + (kk + 1) * 64],
                        start=False, stop=(idx == 3))
                h_prev = hpool.tile([128, 128], f32)
                nc.scalar.activation(h_prev, psum, TANH)
                drain(2)

                pa_t = pab.tile([128, 512], f32, tag="pa")
                for m in range(2):
                    for kk in range(2):
                        nc.tensor.matmul(
                            pa_t[:, m * 64:(m + 1) * 64],
                            lhsT=fc1_s[:, kk * 256 + m * 128:
                                       kk * 256 + (m + 1) * 128],
                            rhs=h_prev[:, kk * 64:(kk + 1) * 64],
                            start=(kk == 0), stop=(kk == 1))
                r_prev = rpool.tile([128, 128], f32)
                for m in range(2):
                    nc.vector.tensor_scalar(
                        r_prev[:, m * 64:(m + 1) * 64],
                        pa_t[:, m * 64:(m + 1) * 64],
                        b1_s[:, m:m + 1], 0.0, ADD, MAX)
                oring2 = oring2p.tile([OD, C2 * B], f32)
                po_t = pob.tile([OD, 512], f32, tag="po")
                for kk in range(2):
                    nc.tensor.matmul(
                        po_t[:, 0:B],
                        lhsT=fc2_s[:, kk * OD:(kk + 1) * OD],
                        rhs=r_prev[:, kk * 64:(kk + 1) * 64],
                        start=(kk == 0), stop=(kk == 1))
                nc.vector.tensor_scalar_add(
                    oring2[:, 0:B], po_t[:, 0:B], b2_s)
                drain(len(pending))

            # ---- autoregressive phase: t in [ps+1, T) ----
            with tc.tile_pool(name="pa2", bufs=2, space="PSUM") as pa2, \
                 tc.tile_pool(name="po2", bufs=2, space="PSUM") as po2:

                fc2_pend = None
                for c in range((T - ps) // C2):
                    t0 = ps + c * C2
                    xa2 = xpool.tile([D - OD + 1, C2 * B], f32, tag="x2")
                    nc.sync.dma_start(
                        out=xa2[1:D - OD + 1, :],
                        in_=xT[OD:D, t0 * B:(t0 + C2) * B])
                    nc.vector.memset(xa2[0:1, :], 1.0)
                    if c > 0:
                        oring2 = oring2p.tile([OD, C2 * B], f32)

                    for j in range(1 if c == 0 else 0, C2):
                        t = t0 + j
                        psum = ph.tile([128, 128], f32)
                        xs = xa2[:, j * B:(j + 1) * B]
                        for m in range(2):
                            nc.tensor.matmul(
                                psum[:, m * 64:(m + 1) * 64],
                                lhsT=wtail_s[:, m * 128:(m + 1) * 128],
                                rhs=xs, start=(m == 0), stop=False)
                        for kk, m in ((0, 0), (1, 0), (0, 1), (1, 1)):
                            nc.tensor.matmul(
                                psum[:, m * 64:(m + 1) * 64],
                                lhsT=whh_s[:, kk * 256 + m * 128:
                                           kk * 256 + (m + 1) * 128],
                                rhs=h_prev[:, kk * 64:(kk + 1) * 64],
                                start=False, stop=False)
                        for idx, (kk, m) in enumerate(
                                ((0, 0), (1, 0), (0, 1), (1, 1))):
                            nc.tensor.matmul(
                                psum[:, m * 64:(m + 1) * 64],
                                lhsT=u_s[:, kk * 256 + m * 128:
                                         kk * 256 + (m + 1) * 128],
                                rhs=r_prev[:, kk * 64:(kk + 1) * 64],
                                start=False, stop=(idx == 3))
                        # previous step's off-path fc2 (keeps PE fed while
                        # this step's tanh completes)
                        if fc2_pend is not None:
                            fc2_pend()
                            fc2_pend = None
                        h_t = hpool.tile([128, 128], f32)
                        nc.scalar.activation(h_t, psum, TANH)
                        pa_t = pa2.tile([128, 128], f32)
                        for m in range(2):
                            for kk in range(2):
                                nc.tensor.matmul(
                                    pa_t[:, m * 64:(m + 1) * 64],
                                    lhsT=fc1_s[:, kk * 256 + m * 128:
                                               kk * 256 + (m + 1) * 128],
                                    rhs=h_t[:, kk * 64:(kk + 1) * 64],
                                    start=(kk == 0), stop=(kk == 1))
                        r_t = rpool.tile([128, 128], f32)
                        for m in range(2):
                            nc.vector.tensor_scalar(
                                r_t[:, m * 64:(m + 1) * 64],
                                pa_t[:, m * 64:(m + 1) * 64],
                                b1_s[:, m:m + 1], 0.0, ADD, MAX)

                        def fc2_emit(r_t=r_t, oring2=oring2, j=j):
                            po_t = po2.tile([OD, B], f32)
                            for kk in range(2):
                                nc.tensor.matmul(
                                    po_t,
                                    lhsT=fc2_s[:, kk * OD:(kk + 1) * OD],
                                    rhs=r_t[:, kk * 64:(kk + 1) * 64],
                                    start=(kk == 0), stop=(kk == 1))
                            nc.vector.tensor_scalar_add(
                                oring2[:, j * B:(j + 1) * B], po_t, b2_s)
                        fc2_pend = fc2_emit
                        h_prev, r_prev = h_t, r_t

                    if fc2_pend is not None:
                        fc2_pend()
                        fc2_pend = None
                    nc.sync.dma_start(
                        out=o_out[:, t0 * B:(t0 + C2) * B], in_=oring2)

    nc.compile()
    return nc


_CACHE = {}
LAST_RESULTS = None


def _get_program(ps):
    if ps not in _CACHE:
        _CACHE[ps] = _build_program(ps)
    return _CACHE[ps]


def _numpy_fallback(x, W_ih, W_hh, b_ih, b_hh, fc1_w, fc1_b, fc2_w, fc2_b, ps):
    b = x.shape[0]
    h = np.zeros((b, H), np.float32)
    o = np.zeros((b, OD), np.float32)
    outs = np.empty((b, T, OD), np.float32)
    for t in range(T):
        if t <= ps:
            x_in = x[:, t, :]
        else:
            x_in = np.concatenate([o, x[:, t, OD:]], axis=-1)
        h = np.tanh(x_in @ W_ih.T + b_ih + h @ W_hh.T + b_hh)
        o = np.maximum(h @ fc1_w.T + fc1_b, 0.0) @ fc2_w.T + fc2_b
        outs[:, t, :] = o
    return outs


def kernel(x, W_ih, W_hh, b_ih, b_hh, fc1_w, fc1_b, fc2_w, fc2_b,
           prediction_start):
    global LAST_RESULTS
    _install_axon_shims()
    x = np.asarray(x, np.float32)
    W_ih = np.asarray(W_ih, np.float32)
    W_hh = np.asarray(W_hh, np.float32)
    b_ih = np.asarray(b_ih, np.float32)
    b_hh = np.asarray(b_hh, np.float32)
    fc1_w = np.asarray(fc1_w, np.float32)
    fc1_b = np.asarray(fc1_b, np.float32)
    fc2_w = np.asarray(fc2_w, np.float32)
    fc2_b = np.asarray(fc2_b, np.float32)
    ps = int(np.asarray(prediction_start))

    if x.shape != (B_FULL, T, D) or ps % C1 != 0 or (T - ps) % C2 != 0 \
            or not (0 < ps < T):
        return _numpy_fallback(x, W_ih, W_hh, b_ih, b_hh, fc1_w, fc1_b,
                               fc2_w, fc2_b, ps)

    from concourse.bass_utils import run_bass_kernel_spmd

    nc = _get_program(ps)

    # ---- host-side packing ----
    bias = (b_ih + b_hh).astype(np.float32)
    U = (W_ih[:, :OD] @ fc2_w).astype(np.float32)  # [H, A]
    bias2 = (bias + W_ih[:, :OD] @ fc2_b).astype(np.float32)

    wih_np = np.concatenate([bias[None, :], W_ih.T], 0)          # [65, 256]
    wtail_np = np.concatenate([bias2[None, :], W_ih[:, OD:].T], 0)  # [49, 256]

    def pack_kxm(Wt):  # Wt: [K=256, M=256] -> [128, 512] blocks (kk, m)
        out = np.empty((128, 512), np.float32)
        for kk in range(2):
            for m in range(2):
                out[:, kk * 256 + m * 128:kk * 256 + (m + 1) * 128] = \
                    Wt[kk * 128:(kk + 1) * 128, m * 128:(m + 1) * 128]
        return out

    whh_np = pack_kxm(W_hh.T)
    fc1_np = pack_kxm(fc1_w.T)
    u_np = pack_kxm(U.T)
    fc2_np = np.empty((128, 2 * OD), np.float32)
    for kk in range(2):
        fc2_np[:, kk * OD:(kk + 1) * OD] = \
            fc2_w.T[kk * 128:(kk + 1) * 128, :]
    b1_np = np.stack([fc1_b[:128], fc1_b[128:]], 1).astype(np.float32)
    b2_np = fc2_b[:, None].astype(np.float32)

    in_maps = []
    for i in range(NCORES):
        xs = x[i * B:(i + 1) * B]                      # [64, T, 64]
        xT_np = np.ascontiguousarray(xs.transpose(2, 1, 0)).reshape(D, T * B)
        in_maps.append({
            "xT": xT_np, "wih": wih_np, "wtail": wtail_np, "whhT": whh_np,
            "fc1T": fc1_np, "uT": u_np, "fc2T": fc2_np, "b1": b1_np,
            "b2": b2_np,
        })

    res = run_bass_kernel_spmd(nc, in_maps, core_ids=list(range(NCORES)))
    LAST_RESULTS = res

    out = np.empty((B_FULL, T, OD), np.float32)
    for i in range(NCORES):
        o = res.results[i]["o_out"].reshape(OD, T, B)
        out[i * B:(i + 1) * B] = o.transpose(2, 1, 0)
    return out
